# revision 15
# baseline (speedup 1.0000x reference)
"""Trainium2 Bass kernel for nn_Actor_GNN (GATv2 x2 + MLP actor head).

Sharding: data-parallel over the 1024 independent graphs -> 128 graphs/core
on 8 NeuronCores.  Per core: 4096 nodes, 32768 edges, edges of graph b are
the contiguous block [b*256,(b+1)*256) and reference nodes [b*32,(b+1)*32).

Strategy: gather/segment-softmax/scatter are expressed as dense matmuls with
one-hot src/dst selection matrices (built host-side, bf16):
  pre[e,:] = S@xl + D@xr + ea@We.T         (TensorE, PSUM accumulate)
  score    = sum_c leakyrelu(pre)*att      (linear part folded into matmul
                                            columns; only att*min(pre,0)
                                            computed on VectorE)
  A_T'[m,(h,n)] = sum_e S[e,m] ex[e,h] D[e,n]   (per-graph 32x32 attention)
  den[n,h] = sum_e D[e,n] ex[e,h]
  out[n,:] = (A_T'_h.T @ xl_h) * (1/den) + bias  (normalize after aggregation)
"""

import sys

sys.path.insert(0, "/opt/trn_rl_repo")

import numpy as np
import ml_dtypes

import concourse.bass as bass
import concourse.mybir as mybir
from concourse import bacc
from concourse import tile
from concourse.bass_utils import run_bass_kernel_spmd
from concourse.masks import make_identity

BF = mybir.dt.bfloat16
F32 = mybir.dt.float32
AF = mybir.ActivationFunctionType
ALU = mybir.AluOpType
bf16 = ml_dtypes.bfloat16

# ---- problem constants (hardcoded) ----
B, NG, F, ED, EG = 1024, 32, 16, 6, 256
H1, C1, C2 = 5, 80, 160
OBS, ACT = 512, 2
NCORES = 8
G = B // NCORES          # 128 graphs per core
NNODES = G * NG          # 4096
NEDGES = G * EG          # 32768
NGRP = NNODES // 128     # 32 groups of 128 nodes (4 graphs)
ETPG = 8                 # edge tiles (of 128) per group (1024 edges)
HC1 = H1 * C1            # 400

_CACHE = {}


def build_nc():
    nc = bacc.Bacc("TRN2", target_bir_lowering=False, debug=False)

    def par(name, shape, dt, out=False):
        return nc.declare_dram_parameter(name, list(shape), dt, isOutput=out)

    d_xT = par("xT", [17, NNODES], BF)            # x.T with ones row 16
    d_ea = par("ea6", [ED, NEDGES], BF)           # edge_attr.T
    d_ST = par("ST", [128, NEDGES], BF)           # src one-hot, node-in-group major
    d_DT = par("DT", [128, NEDGES], BF)           # dst one-hot
    d_Se = par("SeR", [NEDGES // 512, 128, 4, NG], BF)   # src one-hot edge-major
    d_De = par("DeR", [NEDGES // 512, 128, 4, NG], BF)   # dst one-hot edge-major
    d_W1 = par("Wlr1b", [17, 810], BF)            # [Wl1.T|a] , [Wr1.T|a] (+bias row)
    d_We1 = par("We1be", [ED, 405], BF)           # [We1.T | We1.T@att]
    d_att1 = par("att1rep", [128, HC1], BF)       # att1 flat, replicated
    d_bc1 = par("bc1rep", [128, HC1], F32)
    d_W2 = par("Wlr2b", [128, 4, 322], BF)        # chunked [512pad, 322]
    d_We2 = par("We2be", [ED, 161], BF)
    d_att2 = par("att2rep", [128, C2], BF)
    d_bc2 = par("bc2rep", [128, C2], F32)
    d_wd1 = par("Wd1T", [C2, 32], BF)
    d_wd2 = par("Wd2T", [32, OBS], BF)
    d_wf1 = par("Wf1T", [128, 4, 256], BF)        # [512,256] chunked on k
    d_wf2 = par("Wf2T", [128, 2, 256], BF)
    d_wms = par("WmsT", [128, 2, 4], BF)          # [256, 4] chunked
    d_bd1 = par("bd1", [32, 1], F32)
    d_bd2 = par("bd2c", [128, 4], F32)
    d_bf1 = par("bf1c", [128, 2], F32)
    d_bf2 = par("bf2c", [128, 2], F32)
    d_bms = par("bms", [4, 1], F32)
    d_out = par("out", [4, 128], F32, out=True)

    with tile.TileContext(nc) as tc:
        import contextlib

        ctx = contextlib.ExitStack()
        with ctx:
            cpool = ctx.enter_context(tc.tile_pool(name="const", bufs=1))
            persist = ctx.enter_context(tc.tile_pool(name="persist", bufs=1))
            spool = ctx.enter_context(tc.tile_pool(name="stream", bufs=3))
            wpool = ctx.enter_context(tc.tile_pool(name="work", bufs=3))
            npool = ctx.enter_context(tc.tile_pool(name="small", bufs=3))
            ppool = ctx.enter_context(tc.tile_pool(name="psum", bufs=2, space="PSUM"))
            ppool1 = ctx.enter_context(tc.tile_pool(name="psum1", bufs=2, space="PSUM"))

            # ---- constants to SBUF ----
            def cload(dram, shape, dt):
                t = cpool.tile(list(shape), dt, tag=dram.name + "_c")
                nc.sync.dma_start(out=t[...], in_=dram.ap())
                return t

            s_W1 = cload(d_W1, [17, 810], BF)
            s_We1 = cload(d_We1, [ED, 405], BF)
            s_att1 = cload(d_att1, [128, HC1], BF)
            s_bc1 = cload(d_bc1, [128, HC1], F32)
            s_W2 = cload(d_W2, [128, 4, 322], BF)
            s_We2 = cload(d_We2, [ED, 161], BF)
            s_att2 = cload(d_att2, [128, C2], BF)
            s_bc2 = cload(d_bc2, [128, C2], F32)
            s_wd1a = cpool.tile([128, 32], BF, tag="wd1a")
            nc.sync.dma_start(out=s_wd1a[...], in_=d_wd1.ap()[0:128, :])
            s_wd1b = cpool.tile([32, 32], BF, tag="wd1b")
            nc.sync.dma_start(out=s_wd1b[...], in_=d_wd1.ap()[128:160, :])
            s_wd2 = cload(d_wd2, [32, OBS], BF)
            s_wf1 = cload(d_wf1, [128, 4, 256], BF)
            s_wf2 = cload(d_wf2, [128, 2, 256], BF)
            s_wms = cload(d_wms, [128, 2, 4], BF)
            s_bd1 = cload(d_bd1, [32, 1], F32)
            s_bd2 = cload(d_bd2, [128, 4], F32)
            s_bf1 = cload(d_bf1, [128, 2], F32)
            s_bf2 = cload(d_bf2, [128, 2], F32)
            s_bm = cpool.tile([2, 1], F32, tag="bm_c")
            nc.sync.dma_start(out=s_bm[...], in_=d_bms.ap()[0:2, :])
            s_bs = cpool.tile([2, 1], F32, tag="bs_c")
            nc.sync.dma_start(out=s_bs[...], in_=d_bms.ap()[2:4, :])

            ident = cpool.tile([128, 128], BF, tag="ident")
            make_identity(nc, ident[...])

            # ---- persistent activations ----
            xlr = persist.tile([128, NGRP, 810], BF)    # [xl|a1l|xr|a1r] per group
            h1 = persist.tile([128, NGRP, HC1], BF)
            h1T = persist.tile([128, 4, NNODES], BF)    # h1 transposed, c-chunked
            xlr2 = persist.tile([128, NGRP, 322], BF)
            h2 = persist.tile([128, NGRP, C2], BF)
            ego = persist.tile([128, C2], BF)
            egoT0 = persist.tile([128, 128], BF)
            egoT1 = persist.tile([32, 128], BF)
            d1_sb = persist.tile([32, 128], BF)
            d_sb = persist.tile([128, 4, 128], BF)
            f1_sb = persist.tile([128, 2, 128], BF)
            f2_sb = persist.tile([128, 2, 128], BF)
            out_m = persist.tile([2, 128], F32)
            out_s = persist.tile([2, 128], F32)
            ts_sb = persist.tile([2, 128], F32)

            # h1T padding rows: zero tail of chunk 3, ones row 400 (=chunk3 row 16)
            nc.gpsimd.memset(h1T[:, 3, :], 0.0)
            nc.vector.memset(h1T[32:64, 3, :], 1.0)

            # ---- phase A: node transforms layer 1 ----
            for grp in range(NGRP):
                xt = spool.tile([17, 128], BF, tag="xt")
                nc.sync.dma_start(out=xt[...], in_=d_xT.ap()[:, grp * 128:(grp + 1) * 128])
                psA = ppool.tile([128, 405], F32, tag="pre")
                psB = ppool.tile([128, 405], F32, tag="pre")
                nc.tensor.matmul(psA[...], xt[...], s_W1[:, 0:405], start=True, stop=True)
                nc.tensor.matmul(psB[...], xt[...], s_W1[:, 405:810], start=True, stop=True)
                nc.scalar.activation(xlr[:, grp, 0:405], psA[...], AF.Copy)
                nc.scalar.activation(xlr[:, grp, 405:810], psB[...], AF.Copy)

            # ---- per-group main loop ----
            for grp in range(NGRP):
                e0 = grp * 1024  # first edge of group
                st_c, dt_c, se_c, de_c = [], [], [], []
                for c in range(2):
                    stt_ = spool.tile([128, 512], BF, tag="st")
                    dtt_ = spool.tile([128, 512], BF, tag="dt")
                    set_ = spool.tile([128, 4, NG], BF, tag="se")
                    det_ = spool.tile([128, 4, NG], BF, tag="de")
                    sl = slice(e0 + c * 512, e0 + (c + 1) * 512)
                    nc.sync.dma_start(out=stt_[...], in_=d_ST.ap()[:, sl])
                    nc.sync.dma_start(out=dtt_[...], in_=d_DT.ap()[:, sl])
                    nc.sync.dma_start(out=set_[...], in_=d_Se.ap()[grp * 2 + c])
                    nc.sync.dma_start(out=det_[...], in_=d_De.ap()[grp * 2 + c])
                    st_c.append(stt_); dt_c.append(dtt_); se_c.append(set_); de_c.append(det_)
                ea1 = spool.tile([ED, 1024], BF, tag="ea")
                nc.sync.dma_start(out=ea1[...], in_=d_ea.ap()[:, e0:e0 + 1024])

                # ===== layer 1 =====
                score1 = npool.tile([128, ETPG, H1], F32, tag="score")
                for t in range(ETPG):
                    c, jj = t // 4, t % 4
                    pre = ppool.tile([128, 405], F32, tag="pre")
                    nc.tensor.matmul(pre[...], st_c[c][:, jj * 128:(jj + 1) * 128],
                                     xlr[:, grp, 0:405], start=True, stop=False)
                    nc.tensor.matmul(pre[...], dt_c[c][:, jj * 128:(jj + 1) * 128],
                                     xlr[:, grp, 405:810], start=False, stop=False)
                    nc.tensor.matmul(pre[...], ea1[:, t * 128:(t + 1) * 128],
                                     s_We1[...], start=False, stop=True)
                    # score = term1 - 0.8 * sum_c att*min(pre,0)
                    pm = wpool.tile([128, HC1], BF, tag="pm")
                    nc.vector.tensor_scalar_min(pm[...], pre[:, 0:HC1], 0.0)
                    pma = wpool.tile([128, HC1], BF, tag="pma")
                    nc.vector.scalar_tensor_tensor(pma[...], pm[...], -0.8, s_att1[...],
                                                   op0=ALU.mult, op1=ALU.mult)
                    msc = npool.tile([128, H1], F32, tag="msc")
                    nc.vector.tensor_reduce(msc[...], pma[...].rearrange("p (h c) -> p h c", h=H1),
                                            axis=mybir.AxisListType.X, op=ALU.add)
                    nc.vector.scalar_tensor_tensor(score1[:, t, :], msc[...], 0.0,
                                                   pre[:, HC1:405], op0=ALU.bypass, op1=ALU.add)
                ex1 = npool.tile([128, ETPG, H1], BF, tag="ex")
                nc.scalar.activation(ex1[...], score1[...], AF.Exp)

                at1 = ppool1.tile([128, H1, NG], F32, tag="ats")
                den1 = ppool1.tile([128, H1], F32, tag="den")
                for t in range(ETPG):
                    c, jj = t // 4, t % 4
                    g = t // 2
                    first = (t % 2 == 0)
                    exd = wpool.tile([128, H1, NG], BF, tag="exd")
                    nc.vector.scalar_tensor_tensor(
                        exd[...],
                        de_c[c][:, jj, None, :].broadcast_to([128, H1, NG]),
                        0.0,
                        ex1[:, t, :, None].broadcast_to([128, H1, NG]),
                        op0=ALU.bypass, op1=ALU.mult)
                    nc.tensor.matmul(at1[32 * g:32 * (g + 1), :, :], se_c[c][:, jj, :],
                                     exd[...], start=first, stop=not first,
                                     tile_position=(0, 32 * g))
                    nc.tensor.matmul(den1[32 * g:32 * (g + 1), :], de_c[c][:, jj, :],
                                     ex1[:, t, :], start=first, stop=not first,
                                     tile_position=(0, 32 * g))
                at1_sb = wpool.tile([128, H1, NG], BF, tag="at1sb")
                nc.scalar.activation(at1_sb[...], at1[...], AF.Copy)
                den1_sb = npool.tile([128, H1], F32, tag="densb")
                nc.scalar.activation(den1_sb[...], den1[...], AF.Copy, bias=1e-16)
                rec1 = npool.tile([128, H1], F32, tag="rec")
                nc.vector.reciprocal(rec1[...], den1_sb[...])

                o1 = ppool.tile([128, HC1], F32, tag="agg")
                for g in range(4):
                    for h in range(H1):
                        nc.tensor.matmul(
                            o1[32 * g:32 * (g + 1), h * C1:(h + 1) * C1],
                            at1_sb[32 * g:32 * (g + 1), h, :],
                            xlr[32 * g:32 * (g + 1), grp, h * C1:(h + 1) * C1],
                            start=True, stop=True, tile_position=(32 * g, 32 * g))
                t1 = wpool.tile([128, HC1], F32, tag="t1")
                for h in range(H1):
                    hs = slice(h * C1, (h + 1) * C1)
                    nc.vector.scalar_tensor_tensor(t1[:, hs], o1[:, hs], rec1[:, h:h + 1],
                                                   s_bc1[:, hs], op0=ALU.mult, op1=ALU.add)
                nc.scalar.activation(h1[:, grp, :], t1[...], AF.Relu)

                # ---- transpose h1(grp) into h1T ----
                trp = ppool.tile([128, 4, 128], BF, tag="agg")
                for j in range(4):
                    csz = 128 if j < 3 else 16
                    nc.tensor.transpose(trp[0:csz, j, :], h1[:, grp, j * 128:j * 128 + csz],
                                        ident[...])
                for j in range(4):
                    csz = 128 if j < 3 else 16
                    nc.vector.tensor_copy(h1T[0:csz, j, grp * 128:(grp + 1) * 128],
                                          trp[0:csz, j, :])

                # ---- layer-2 node transform for this group ----
                ps2 = ppool.tile([128, 322], F32, tag="pre")
                for j in range(4):
                    nc.tensor.matmul(ps2[...], h1T[:, j, grp * 128:(grp + 1) * 128],
                                     s_W2[:, j, :], start=(j == 0), stop=(j == 3))
                nc.scalar.activation(xlr2[:, grp, :], ps2[...], AF.Copy)

                # ===== layer 2 =====
                score2 = npool.tile([128, ETPG], F32, tag="score")
                for t in range(ETPG):
                    c, jj = t // 4, t % 4
                    pre2 = ppool.tile([128, 161], F32, tag="pre")
                    nc.tensor.matmul(pre2[...], st_c[c][:, jj * 128:(jj + 1) * 128],
                                     xlr2[:, grp, 0:161], start=True, stop=False)
                    nc.tensor.matmul(pre2[...], dt_c[c][:, jj * 128:(jj + 1) * 128],
                                     xlr2[:, grp, 161:322], start=False, stop=False)
                    nc.tensor.matmul(pre2[...], ea1[:, t * 128:(t + 1) * 128],
                                     s_We2[...], start=False, stop=True)
                    pm2 = wpool.tile([128, C2], BF, tag="pm")
                    nc.vector.tensor_scalar_min(pm2[...], pre2[:, 0:C2], 0.0)
                    pma2 = wpool.tile([128, C2], BF, tag="pma")
                    nc.vector.scalar_tensor_tensor(pma2[...], pm2[...], -0.8, s_att2[...],
                                                   op0=ALU.mult, op1=ALU.mult)
                    msc2 = npool.tile([128, 1], F32, tag="msc")
                    nc.vector.tensor_reduce(msc2[...], pma2[...], axis=mybir.AxisListType.X,
                                            op=ALU.add)
                    nc.vector.scalar_tensor_tensor(score2[:, t:t + 1], msc2[...], 0.0,
                                                   pre2[:, C2:161], op0=ALU.bypass, op1=ALU.add)
                ex2 = npool.tile([128, ETPG], BF, tag="ex")
                nc.scalar.activation(ex2[...], score2[...], AF.Exp)

                at2 = ppool1.tile([128, NG], F32, tag="ats")
                den2 = ppool1.tile([128, 1], F32, tag="den")
                for t in range(ETPG):
                    c, jj = t // 4, t % 4
                    g = t // 2
                    first = (t % 2 == 0)
                    exd2 = wpool.tile([128, NG], BF, tag="exd")
                    nc.vector.scalar_tensor_tensor(
                        exd2[...], de_c[c][:, jj, :], 0.0,
                        ex2[:, t:t + 1].broadcast_to([128, NG]),
                        op0=ALU.bypass, op1=ALU.mult)
                    nc.tensor.matmul(at2[32 * g:32 * (g + 1), :], se_c[c][:, jj, :],
                                     exd2[...], start=first, stop=not first,
                                     tile_position=(0, 32 * g))
                    nc.tensor.matmul(den2[32 * g:32 * (g + 1), :], de_c[c][:, jj, :],
                                     ex2[:, t:t + 1], start=first, stop=not first,
                                     tile_position=(0, 32 * g))
                at2_sb = wpool.tile([128, NG], BF, tag="at2sb")
                nc.scalar.activation(at2_sb[...], at2[...], AF.Copy)
                den2_sb = npool.tile([128, 1], F32, tag="densb")
                nc.scalar.activation(den2_sb[...], den2[...], AF.Copy, bias=1e-16)
                rec2 = npool.tile([128, 1], F32, tag="rec")
                nc.vector.reciprocal(rec2[...], den2_sb[...])

                o2 = ppool.tile([128, C2], F32, tag="agg")
                for g in range(4):
                    nc.tensor.matmul(o2[32 * g:32 * (g + 1), :], at2_sb[32 * g:32 * (g + 1), :],
                                     xlr2[32 * g:32 * (g + 1), grp, 0:C2],
                                     start=True, stop=True, tile_position=(32 * g, 32 * g))
                t2 = wpool.tile([128, C2], F32, tag="t1")
                nc.vector.scalar_tensor_tensor(t2[...], o2[...], rec2[:, 0:1], s_bc2[...],
                                               op0=ALU.mult, op1=ALU.add)
                nc.scalar.activation(h2[:, grp, :], t2[...], AF.Relu)

            # ---- ego extraction: graph g at partition (g%32)*4 + g//32 ----
            for a in range(4):
                nc.sync.dma_start(out=ego[a * 32:(a + 1) * 32, :], in_=h2[32 * a:32 * a + 1, :, :])
            tpa = ppool.tile([128, 128], BF, tag="agg")
            nc.tensor.transpose(tpa[...], ego[:, 0:128], ident[...])
            nc.scalar.activation(egoT0[...], tpa[...], AF.Copy)
            tpb = ppool.tile([32, 128], BF, tag="agg")
            nc.tensor.transpose(tpb[...], ego[:, 128:160], ident[...])
            nc.scalar.activation(egoT1[...], tpb[...], AF.Copy)

            # ---- head MLP (feature-major, batch=128 on free dim) ----
            d1p = ppool.tile([32, 128], F32, tag="agg")
            nc.tensor.matmul(d1p[...], s_wd1a[...], egoT0[...], start=True, stop=False)
            nc.tensor.matmul(d1p[...], s_wd1b[...], egoT1[...], start=False, stop=True)
            nc.scalar.activation(d1_sb[...], d1p[...], AF.Identity, bias=s_bd1[...])
            for j in range(4):
                dp = ppool.tile([128, 128], F32, tag="agg")
                nc.tensor.matmul(dp[...], s_wd2[:, j * 128:(j + 1) * 128], d1_sb[...],
                                 start=True, stop=True)
                nc.scalar.activation(d_sb[:, j, :], dp[...], AF.Tanh, bias=s_bd2[:, j:j + 1])
            for m in range(2):
                fp = ppool.tile([128, 128], F32, tag="agg")
                for j in range(4):
                    nc.tensor.matmul(fp[...], s_wf1[:, j, m * 128:(m + 1) * 128], d_sb[:, j, :],
                                     start=(j == 0), stop=(j == 3))
                nc.scalar.activation(f1_sb[:, m, :], fp[...], AF.Relu, bias=s_bf1[:, m:m + 1])
            for m in range(2):
                fp2 = ppool.tile([128, 128], F32, tag="agg")
                for j in range(2):
                    nc.tensor.matmul(fp2[...], s_wf2[:, j, m * 128:(m + 1) * 128], f1_sb[:, j, :],
                                     start=(j == 0), stop=(j == 1))
                nc.scalar.activation(f2_sb[:, m, :], fp2[...], AF.Relu, bias=s_bf2[:, m:m + 1])
            msp = ppool.tile([2, 128], F32, tag="agg")
            ssp = ppool.tile([2, 128], F32, tag="agg")
            for j in range(2):
                nc.tensor.matmul(msp[...], s_wms[:, j, 0:2], f2_sb[:, j, :],
                                 start=(j == 0), stop=(j == 1))
            for j in range(2):
                nc.tensor.matmul(ssp[...], s_wms[:, j, 2:4], f2_sb[:, j, :],
                                 start=(j == 0), stop=(j == 1))
            nc.scalar.activation(out_m[...], msp[...], AF.Identity, bias=s_bm[...])
            nc.scalar.activation(ts_sb[...], ssp[...], AF.Tanh, bias=s_bs[...])
            nc.scalar.activation(out_s[...], ts_sb[...], AF.Copy, scale=3.5, bias=-1.5)
            nc.sync.dma_start(out=d_out.ap()[0:2, :], in_=out_m[...])
            nc.sync.dma_start(out=d_out.ap()[2:4, :], in_=out_s[...])

    nc.compile()
    return nc


def _prep_core_inputs(inputs, c):
    """Host-side preprocessing for core c (all free / untimed)."""
    ns = slice(c * NNODES, (c + 1) * NNODES)
    es = slice(c * NEDGES, (c + 1) * NEDGES)
    x = np.asarray(inputs["x"])[ns]                       # [4096, 16]
    ea = np.asarray(inputs["edge_attr"])[es]              # [32768, 6]
    src = np.asarray(inputs["edge_index"])[0, es] - c * NNODES
    dst = np.asarray(inputs["edge_index"])[1, es] - c * NNODES

    e = np.arange(NEDGES)
    ST = np.zeros((128, NEDGES), np.float32)
    DT = np.zeros((128, NEDGES), np.float32)
    ST[src % 128, e] = 1.0
    DT[dst % 128, e] = 1.0
    Se = np.zeros((NEDGES, NG), np.float32)
    De = np.zeros((NEDGES, NG), np.float32)
    Se[e, src % NG] = 1.0
    De[e, dst % NG] = 1.0
    SeR = Se.reshape(64, 4, 128, NG).transpose(0, 2, 1, 3)
    DeR = De.reshape(64, 4, 128, NG).transpose(0, 2, 1, 3)

    xT = np.concatenate([x.T, np.ones((1, NNODES), np.float32)], 0)  # [17, 4096]

    return {
        "xT": xT.astype(bf16),
        "ea6": ea.T.astype(bf16).copy(),
        "ST": ST.astype(bf16),
        "DT": DT.astype(bf16),
        "SeR": np.ascontiguousarray(SeR).astype(bf16),
        "DeR": np.ascontiguousarray(DeR).astype(bf16),
    }


def _prep_weights(inputs):
    ii = {k: np.asarray(v).astype(np.float32) for k, v in inputs.items()
          if k not in ("x", "edge_index", "edge_attr")}
    att1 = ii["att1"]                                     # [5, 80]
    att1f = att1.reshape(-1)                              # [400]
    # Wlr1b [17, 810]: [Wl1.T|Wl1att] [Wr1.T|Wr1att] with bias row 16
    W1 = np.zeros((17, 810), np.float32)
    W1[0:16, 0:400] = ii["Wl1"].T
    W1[16, 0:400] = ii["bl1"]
    for h in range(H1):
        W1[0:16, 400 + h] = ii["Wl1"].T[:, h * C1:(h + 1) * C1] @ att1[h]
        W1[16, 400 + h] = ii["bl1"][h * C1:(h + 1) * C1] @ att1[h]
    W1[0:16, 405:805] = ii["Wr1"].T
    W1[16, 405:805] = ii["br1"]
    for h in range(H1):
        W1[0:16, 805 + h] = ii["Wr1"].T[:, h * C1:(h + 1) * C1] @ att1[h]
        W1[16, 805 + h] = ii["br1"][h * C1:(h + 1) * C1] @ att1[h]
    We1 = np.zeros((ED, 405), np.float32)
    We1[:, 0:400] = ii["We1"].T
    for h in range(H1):
        We1[:, 400 + h] = ii["We1"].T[:, h * C1:(h + 1) * C1] @ att1[h]

    att2 = ii["att2"].reshape(-1)                         # [160]
    W2 = np.zeros((512, 322), np.float32)
    W2[0:400, 0:160] = ii["Wl2"].T
    W2[0:400, 160] = ii["Wl2"].T @ att2
    W2[0:400, 161:321] = ii["Wr2"].T
    W2[0:400, 321] = ii["Wr2"].T @ att2
    W2[416, 0:160] = ii["bl2"]
    W2[416, 160] = ii["bl2"] @ att2
    W2[416, 161:321] = ii["br2"]
    W2[416, 321] = ii["br2"] @ att2
    We2 = np.zeros((ED, 161), np.float32)
    We2[:, 0:160] = ii["We2"].T
    We2[:, 160] = ii["We2"].T @ att2

    Wf1 = ii["Wf1"].T.reshape(4, 128, 256).copy()         # [512,256] -> k-chunks
    Wf2 = ii["Wf2"].T.reshape(2, 128, 256).copy()
    Wms = np.concatenate([ii["Wm"].T, ii["Ws"].T], 1)     # [256, 4]
    Wmsr = Wms.reshape(2, 128, 4).copy()

    return {
        "Wlr1b": W1.astype(bf16),
        "We1be": We1.astype(bf16),
        "att1rep": np.broadcast_to(att1f, (128, 400)).astype(bf16).copy(),
        "bc1rep": np.broadcast_to(ii["bc1"], (128, 400)).astype(np.float32).copy(),
        "Wlr2b": W2.reshape(4, 128, 322).transpose(1, 0, 2).astype(bf16).copy(),
        "We2be": We2.astype(bf16),
        "att2rep": np.broadcast_to(att2, (128, 160)).astype(bf16).copy(),
        "bc2rep": np.broadcast_to(ii["bc2"], (128, 160)).astype(np.float32).copy(),
        "Wd1T": ii["Wd1"].T.astype(bf16).copy(),
        "Wd2T": ii["Wd2"].T.astype(bf16).copy(),
        "Wf1T": np.transpose(Wf1, (1, 0, 2)).astype(bf16).copy(),
        "Wf2T": np.transpose(Wf2, (1, 0, 2)).astype(bf16).copy(),
        "WmsT": np.transpose(Wmsr, (1, 0, 2)).astype(bf16).copy(),
        "bd1": ii["bd1"][:, None].astype(np.float32).copy(),
        "bd2c": ii["bd2"].reshape(4, 128).T.astype(np.float32).copy(),
        "bf1c": ii["bf1"].reshape(2, 128).T.astype(np.float32).copy(),
        "bf2c": ii["bf2"].reshape(2, 128).T.astype(np.float32).copy(),
        "bms": np.concatenate([ii["bm"], ii["bs"]])[:, None].astype(np.float32).copy(),
    }


LAST = {}


def kernel(**inputs):
    if "nc" not in _CACHE:
        _CACHE["nc"] = build_nc()
    nc = _CACHE["nc"]

    wts = _prep_weights(inputs)
    in_maps = []
    for c in range(NCORES):
        m = dict(wts)
        m.update(_prep_core_inputs(inputs, c))
        in_maps.append(m)

    res = run_bass_kernel_spmd(nc, in_maps, core_ids=list(range(NCORES)),
                               trace=LAST.get("trace", False))
    LAST["exec_time_ns"] = res.exec_time_ns
    LAST["res"] = res

    mean = np.zeros((B, ACT), np.float32)
    logstd = np.zeros((B, ACT), np.float32)
    p = np.arange(128)
    gperm = (p % 32) * 4 + p // 32        # partition p holds graph gperm[p]
    for c in range(NCORES):
        o = res.results[c]["out"]          # [4, 128]
        mean[c * G + gperm, :] = o[0:2, :].T
        logstd[c * G + gperm, :] = o[2:4, :].T
    return mean, logstd
